# revision 5
# baseline (speedup 1.0000x reference)
"""AdaFace loss kernel for 8 TRN2 NeuronCores.

Math notes (reference is AdaFace with T_ALPHA=1, labels all valid):
  - Off-label columns: cos(clip(arccos(x), eps, pi-eps)) == min(x, cos(eps))
    exactly for x in [0, 1), so the [N, C] bulk is one dual-op
    tensor_scalar pass (min with cos(eps), then mult by S).
  - Label column per row: with theta = arccos(xl), g = -M*ms,
    cos(theta + g) = xl*cos(g) - sqrt(1-xl^2)*sin(g).  The lower clip
    (theta+g < eps -> eps) triggers iff eps-g > 0 AND xl > cos(eps-g);
    cos(eps-g) = ce*cos(g) + se*sin(g).  Upper clip can't trigger.
    Final label value: S * (clip(cos_m, -ce, ce) - (M + M*ms)).
  - Sharding: C split across 8 cores (6250 cols each); norms/labels are
    tiny and replicated so batch stats are computed redundantly per core
    (no collectives).  Label fix-ups applied with indirect DMA
    gather/scatter using flat offsets; rows whose label falls outside a
    core's shard get a huge sentinel offset and are skipped via the
    bounds check.
"""

import math

import numpy as np

N = 2048
C = 50000
NCORES = 8
CS = C // NCORES  # 6250 columns per core
P = 128
RB = N // P  # 16 row blocks

M = 0.4
H = 0.333
S = 64.0
EPS = 1e-3

CE = float(np.cos(np.float32(EPS), dtype=np.float32))  # cos(eps) in f32
SE = float(np.sin(np.float32(EPS), dtype=np.float32))  # sin(eps) in f32
SENTINEL = np.int32(1 << 30)

_COMPILED = None


def _build():
    import sys

    if "/opt/trn_rl_repo" not in sys.path:
        sys.path.insert(0, "/opt/trn_rl_repo")

    import concourse.bass as bass
    import concourse.tile as tile
    from concourse import bacc, bass_isa, mybir

    f32 = mybir.dt.float32
    i32 = mybir.dt.int32
    Alu = mybir.AluOpType
    Act = mybir.ActivationFunctionType

    nc = bacc.Bacc(
        "TRN2",
        target_bir_lowering=False,
        debug=False,
        enable_asserts=False,
        num_devices=NCORES,
    )

    cos_in = nc.dram_tensor("cosine", [N, CS], f32, kind="ExternalInput")
    norms_t = nc.dram_tensor("norms_t", [P, RB], f32, kind="ExternalInput")
    off_t = nc.dram_tensor("off", [P, RB], i32, kind="ExternalInput")
    out_t = nc.dram_tensor("out", [N, CS], f32, kind="ExternalOutput")

    with tile.TileContext(nc) as tc:
        with (
            tc.tile_pool(name="small", bufs=1) as sp,
            tc.tile_pool(name="stream", bufs=3) as stp,
        ):
            # ---- per-row margin scalars (all [P, RB]; row r = rb*128 + p) ----
            nt = sp.tile([P, RB], f32)
            nc.sync.dma_start(out=nt[:], in_=norms_t.ap())
            off = sp.tile([P, RB], i32)
            nc.sync.dma_start(out=off[:], in_=off_t.ap())

            n_c = sp.tile([P, RB], f32)
            nc.vector.tensor_scalar(
                out=n_c[:], in0=nt[:], scalar1=EPS, scalar2=100.0,
                op0=Alu.max, op1=Alu.min,
            )

            # mean over all 2048 rows (partition all-reduce + free reduce)
            ar1 = sp.tile([P, RB], f32)
            nc.gpsimd.partition_all_reduce(ar1[:], n_c[:], P, bass_isa.ReduceOp.add)
            mean = sp.tile([P, 1], f32)
            nc.vector.tensor_reduce(
                out=mean[:], in_=ar1[:], axis=mybir.AxisListType.X, op=Alu.add
            )
            nc.vector.tensor_scalar(
                out=mean[:], in0=mean[:], scalar1=1.0 / N, scalar2=None, op0=Alu.mult
            )

            diff = sp.tile([P, RB], f32)
            nc.vector.tensor_scalar(
                out=diff[:], in0=n_c[:], scalar1=mean[:, :1], scalar2=None,
                op0=Alu.subtract,
            )
            sq = sp.tile([P, RB], f32)
            nc.vector.tensor_tensor(out=sq[:], in0=diff[:], in1=diff[:], op=Alu.mult)
            ar2 = sp.tile([P, RB], f32)
            nc.gpsimd.partition_all_reduce(ar2[:], sq[:], P, bass_isa.ReduceOp.add)
            var = sp.tile([P, 1], f32)
            nc.vector.tensor_reduce(
                out=var[:], in_=ar2[:], axis=mybir.AxisListType.X, op=Alu.add
            )
            nc.vector.tensor_scalar(
                out=var[:], in0=var[:], scalar1=1.0 / (N - 1), scalar2=None,
                op0=Alu.mult,
            )
            std = sp.tile([P, 1], f32)
            nc.scalar.activation(std[:], var[:], Act.Sqrt)
            nc.vector.tensor_scalar(
                out=std[:], in0=std[:], scalar1=EPS, scalar2=None, op0=Alu.add
            )
            dinv = sp.tile([P, 1], f32)
            nc.vector.reciprocal(dinv[:], std[:])

            ms = sp.tile([P, RB], f32)
            nc.vector.tensor_scalar(
                out=ms[:], in0=diff[:], scalar1=dinv[:, :1], scalar2=H,
                op0=Alu.mult, op1=Alu.mult,
            )
            nc.vector.tensor_scalar(
                out=ms[:], in0=ms[:], scalar1=-1.0, scalar2=1.0,
                op0=Alu.max, op1=Alu.min,
            )

            # g = -M*ms;  sin(g), cos(g) via the Sin LUT (cos is even)
            halfpi = sp.tile([P, 1], f32)
            nc.vector.memset(halfpi[:], math.pi / 2)
            sin_g = sp.tile([P, RB], f32)
            nc.scalar.activation(sin_g[:], ms[:], Act.Sin, scale=-M)
            cos_g = sp.tile([P, RB], f32)
            nc.scalar.activation(
                cos_g[:], ms[:], Act.Sin, scale=-M, bias=halfpi[:, :1]
            )

            # ---- gather xl = cosine[r, label[r]] (local shard only) ----
            # One call per rb column: the HW DGE emits one descriptor per
            # partition, moving that partition's whole free-dim run and
            # consuming one offset per partition — so gather/scatter must
            # use [128, 1] data+offset slices to move exactly one element
            # per row.
            xl = sp.tile([P, RB], f32)
            nc.vector.memset(xl[:], 0.0)
            for rb in range(RB):
                nc.gpsimd.indirect_dma_start(
                    out=xl[:, rb : rb + 1],
                    out_offset=None,
                    in_=cos_in.ap(),
                    in_offset=bass.IndirectOffsetOnAxis(
                        ap=off[:, rb : rb + 1], axis=1
                    ),
                    bounds_check=N * CS - 1,
                    oob_is_err=False,
                )

            # s = sqrt(1 - xl^2)
            xsq = sp.tile([P, RB], f32)
            nc.scalar.activation(xsq[:], xl[:], Act.Square)
            sroot = sp.tile([P, RB], f32)
            nc.scalar.activation(sroot[:], xsq[:], Act.Sqrt, scale=-1.0, bias=1.0)

            # cos_m = xl*cos_g - s*sin_g
            ta = sp.tile([P, RB], f32)
            nc.vector.tensor_tensor(out=ta[:], in0=xl[:], in1=cos_g[:], op=Alu.mult)
            tb = sp.tile([P, RB], f32)
            nc.vector.tensor_tensor(out=tb[:], in0=sroot[:], in1=sin_g[:], op=Alu.mult)
            cosm = sp.tile([P, RB], f32)
            nc.vector.tensor_tensor(out=cosm[:], in0=ta[:], in1=tb[:], op=Alu.subtract)

            # lower-clip case: theta+g < eps  <=>  ms > -EPS/M  AND  xl > cos(eps-g)
            m1 = sp.tile([P, RB], f32)
            nc.vector.tensor_scalar(
                out=m1[:], in0=ms[:], scalar1=-EPS / M, scalar2=None, op0=Alu.is_gt
            )
            t1 = sp.tile([P, RB], f32)
            nc.vector.tensor_scalar(
                out=t1[:], in0=cos_g[:], scalar1=CE, scalar2=None, op0=Alu.mult
            )
            t2 = sp.tile([P, RB], f32)
            nc.vector.tensor_scalar(
                out=t2[:], in0=sin_g[:], scalar1=SE, scalar2=None, op0=Alu.mult
            )
            thresh = sp.tile([P, RB], f32)
            nc.vector.tensor_tensor(out=thresh[:], in0=t1[:], in1=t2[:], op=Alu.add)
            m2 = sp.tile([P, RB], f32)
            nc.vector.tensor_tensor(out=m2[:], in0=xl[:], in1=thresh[:], op=Alu.is_gt)
            maskc = sp.tile([P, RB], f32)
            nc.vector.tensor_tensor(out=maskc[:], in0=m1[:], in1=m2[:], op=Alu.mult)
            # cosm = cosm + mask * (CE - cosm)
            dce = sp.tile([P, RB], f32)
            nc.vector.tensor_scalar(
                out=dce[:], in0=cosm[:], scalar1=-1.0, scalar2=CE,
                op0=Alu.mult, op1=Alu.add,
            )
            mce = sp.tile([P, RB], f32)
            nc.vector.tensor_tensor(out=mce[:], in0=maskc[:], in1=dce[:], op=Alu.mult)
            nc.vector.tensor_tensor(out=cosm[:], in0=cosm[:], in1=mce[:], op=Alu.add)

            # fixv = S*(clip(cosm, -ce, ce) - M - M*ms)
            v = sp.tile([P, RB], f32)
            nc.vector.tensor_scalar(
                out=v[:], in0=cosm[:], scalar1=-CE, scalar2=CE,
                op0=Alu.max, op1=Alu.min,
            )
            q = sp.tile([P, RB], f32)
            nc.vector.tensor_scalar(
                out=q[:], in0=v[:], scalar1=S, scalar2=-S * M,
                op0=Alu.mult, op1=Alu.add,
            )
            r_ = sp.tile([P, RB], f32)
            nc.vector.tensor_scalar(
                out=r_[:], in0=ms[:], scalar1=S * M, scalar2=None, op0=Alu.mult
            )
            fixv = sp.tile([P, RB], f32)
            nc.vector.tensor_tensor(out=fixv[:], in0=q[:], in1=r_[:], op=Alu.subtract)

            # ---- streaming bulk pass: out = min(x, ce) * S ----
            for rb in range(RB):
                t = stp.tile([P, CS], f32)
                rows = slice(rb * P, (rb + 1) * P)
                nc.sync.dma_start(out=t[:], in_=cos_in.ap()[rows, :])
                nc.vector.tensor_scalar(
                    out=t[:], in0=t[:], scalar1=CE, scalar2=S,
                    op0=Alu.min, op1=Alu.mult,
                )
                nc.scalar.dma_start(out=out_t.ap()[rows, :], in_=t[:])

            # ---- scatter label fix-ups (Tile orders these after the stores) ----
            for rb in range(RB):
                nc.gpsimd.indirect_dma_start(
                    out=out_t.ap(),
                    out_offset=bass.IndirectOffsetOnAxis(
                        ap=off[:, rb : rb + 1], axis=1
                    ),
                    in_=fixv[:, rb : rb + 1],
                    in_offset=None,
                    bounds_check=N * CS - 1,
                    oob_is_err=False,
                )

    nc.compile()
    return nc


def _get_compiled():
    global _COMPILED
    if _COMPILED is None:
        _COMPILED = _build()
    return _COMPILED


def _make_in_maps(cosine, norms, label):
    cos = np.ascontiguousarray(np.asarray(cosine, dtype=np.float32))
    nr = np.asarray(norms, dtype=np.float32).reshape(-1)
    lab = np.asarray(label).astype(np.int64).reshape(-1)
    assert cos.shape == (N, C) and nr.shape == (N,) and lab.shape == (N,)

    # row r = rb*128 + p  ->  table[p, rb]
    norms_tab = np.ascontiguousarray(nr.reshape(RB, P).T)

    rows = np.arange(N, dtype=np.int64)
    in_maps = []
    for i in range(NCORES):
        c0 = i * CS
        owned = (lab != -1) & (lab >= c0) & (lab < c0 + CS)
        offv = np.where(owned, rows * CS + (lab - c0), np.int64(SENTINEL)).astype(
            np.int32
        )
        off_tab = np.ascontiguousarray(offv.reshape(RB, P).T)
        in_maps.append(
            {
                "cosine": np.ascontiguousarray(cos[:, c0 : c0 + CS]),
                "norms_t": norms_tab,
                "off": off_tab,
            }
        )
    return in_maps


def _run(in_maps, trace=False, **kwargs):
    import sys

    if "/opt/trn_rl_repo" not in sys.path:
        sys.path.insert(0, "/opt/trn_rl_repo")
    from concourse.bass_utils import run_bass_kernel_spmd

    nc = _get_compiled()
    return run_bass_kernel_spmd(
        nc, in_maps, core_ids=list(range(NCORES)), trace=trace, **kwargs
    )


def kernel(cosine, norms, label):
    in_maps = _make_in_maps(cosine, norms, label)
    res = _run(in_maps)
    outs = [res.results[i]["out"] for i in range(NCORES)]
    return np.concatenate(outs, axis=1)


# revision 11
# speedup vs baseline: 1.1375x; 1.1375x over previous
"""AdaFace loss kernel for 8 TRN2 NeuronCores.

Math notes (reference is AdaFace with T_ALPHA=1, labels all valid):
  - Off-label columns: cos(clip(arccos(x), eps, pi-eps)) == min(x, cos(eps))
    exactly for x in [0, 1), so the [N, C] bulk is one dual-op
    tensor_scalar pass (min with cos(eps), then mult by S).
  - Label column per row: with theta = arccos(xl), g = -M*ms,
    cos(theta + g) = xl*cos(g) - sqrt(1-xl^2)*sin(g).  The lower clip
    (theta+g < eps -> eps) triggers iff eps-g > 0 AND xl > cos(eps-g);
    cos(eps-g) = ce*cos(g) + se*sin(g).  Upper clip can't trigger.
    Final label value: S * (clip(cos_m, -ce, ce) - (M + M*ms)).
  - Sharding: C split across 8 cores (6250 cols each); norms/labels are
    tiny and replicated so batch stats are computed redundantly per core
    (no collectives).  Label fix-ups applied with indirect DMA
    gather/scatter using flat offsets; rows whose label falls outside a
    core's shard get a huge sentinel offset and are skipped via the
    bounds check.
"""

import math

import numpy as np

N = 2048
C = 50000
NCORES = 8
CS = C // NCORES  # 6250 columns per core
P = 128
RB = N // P  # 16 row blocks

M = 0.4
H = 0.333
S = 64.0
EPS = 1e-3

CE = float(np.cos(np.float32(EPS), dtype=np.float32))  # cos(eps) in f32
SE = float(np.sin(np.float32(EPS), dtype=np.float32))  # sin(eps) in f32
SENTINEL = np.int32(1 << 30)

_COMPILED = {}


def _build(k_cols):
    import sys

    if "/opt/trn_rl_repo" not in sys.path:
        sys.path.insert(0, "/opt/trn_rl_repo")

    import concourse.bass as bass
    import concourse.tile as tile
    from concourse import bacc, bass_isa, mybir

    f32 = mybir.dt.float32
    i32 = mybir.dt.int32
    Alu = mybir.AluOpType
    Act = mybir.ActivationFunctionType

    nc = bacc.Bacc(
        "TRN2",
        target_bir_lowering=False,
        debug=False,
        enable_asserts=False,
        num_devices=NCORES,
    )

    cos_in = nc.dram_tensor("cosine", [N, CS], f32, kind="ExternalInput")
    norms_t = nc.dram_tensor("norms_t", [P, RB], f32, kind="ExternalInput")
    off_t = nc.dram_tensor("off", [P, RB], i32, kind="ExternalInput")
    out_t = nc.dram_tensor("out", [N, CS], f32, kind="ExternalOutput")

    with tile.TileContext(nc) as tc:
        with (
            tc.tile_pool(name="small", bufs=1) as sp,
            tc.tile_pool(name="stream", bufs=4) as stp,
        ):
            # ---- per-row margin scalars (all [P, RB]; row r = rb*128 + p) ----
            nt = sp.tile([P, RB], f32)
            nc.sync.dma_start(out=nt[:], in_=norms_t.ap())
            off = sp.tile([P, RB], i32)
            nc.sync.dma_start(out=off[:], in_=off_t.ap())

            n_c = sp.tile([P, RB], f32)
            nc.vector.tensor_scalar(
                out=n_c[:], in0=nt[:], scalar1=EPS, scalar2=100.0,
                op0=Alu.max, op1=Alu.min,
            )

            # mean over all 2048 rows (partition all-reduce + free reduce)
            ar1 = sp.tile([P, RB], f32)
            nc.gpsimd.partition_all_reduce(ar1[:], n_c[:], P, bass_isa.ReduceOp.add)
            mean = sp.tile([P, 1], f32)
            nc.vector.tensor_reduce(
                out=mean[:], in_=ar1[:], axis=mybir.AxisListType.X, op=Alu.add
            )
            nc.vector.tensor_scalar(
                out=mean[:], in0=mean[:], scalar1=1.0 / N, scalar2=None, op0=Alu.mult
            )

            diff = sp.tile([P, RB], f32)
            nc.vector.tensor_scalar(
                out=diff[:], in0=n_c[:], scalar1=mean[:, :1], scalar2=None,
                op0=Alu.subtract,
            )
            sq = sp.tile([P, RB], f32)
            nc.vector.tensor_tensor(out=sq[:], in0=diff[:], in1=diff[:], op=Alu.mult)
            ar2 = sp.tile([P, RB], f32)
            nc.gpsimd.partition_all_reduce(ar2[:], sq[:], P, bass_isa.ReduceOp.add)
            var = sp.tile([P, 1], f32)
            nc.vector.tensor_reduce(
                out=var[:], in_=ar2[:], axis=mybir.AxisListType.X, op=Alu.add
            )
            nc.vector.tensor_scalar(
                out=var[:], in0=var[:], scalar1=1.0 / (N - 1), scalar2=None,
                op0=Alu.mult,
            )
            std = sp.tile([P, 1], f32)
            nc.scalar.activation(std[:], var[:], Act.Sqrt)
            nc.vector.tensor_scalar(
                out=std[:], in0=std[:], scalar1=EPS, scalar2=None, op0=Alu.add
            )
            dinv = sp.tile([P, 1], f32)
            nc.vector.reciprocal(dinv[:], std[:])

            ms = sp.tile([P, RB], f32)
            nc.vector.tensor_scalar(
                out=ms[:], in0=diff[:], scalar1=dinv[:, :1], scalar2=H,
                op0=Alu.mult, op1=Alu.mult,
            )
            nc.vector.tensor_scalar(
                out=ms[:], in0=ms[:], scalar1=-1.0, scalar2=1.0,
                op0=Alu.max, op1=Alu.min,
            )

            # g = -M*ms;  sin(g), cos(g) via the Sin LUT (cos is even)
            halfpi = sp.tile([P, 1], f32)
            nc.vector.memset(halfpi[:], math.pi / 2)
            sin_g = sp.tile([P, RB], f32)
            nc.scalar.activation(sin_g[:], ms[:], Act.Sin, scale=-M)
            cos_g = sp.tile([P, RB], f32)
            nc.scalar.activation(
                cos_g[:], ms[:], Act.Sin, scale=-M, bias=halfpi[:, :1]
            )

            # ---- gather xl = cosine[r, label[r]] (local shard only) ----
            # One call per rb column: the HW DGE emits one descriptor per
            # partition, moving that partition's whole free-dim run and
            # consuming one offset per partition — so gather/scatter must
            # use [128, 1] data+offset slices to move exactly one element
            # per row.
            # Rows are slot-permuted on the host so all rows whose label
            # falls in this core's shard live in the first k_cols columns
            # — only those columns need gather/scatter calls.
            xl = sp.tile([P, RB], f32)
            nc.vector.memset(xl[:], 0.0)
            for j in range(k_cols):
                nc.gpsimd.indirect_dma_start(
                    out=xl[:, j : j + 1],
                    out_offset=None,
                    in_=cos_in.ap(),
                    in_offset=bass.IndirectOffsetOnAxis(
                        ap=off[:, j : j + 1], axis=1
                    ),
                    bounds_check=N * CS - 1,
                    oob_is_err=False,
                )

            # s = sqrt(1 - xl^2)
            xsq = sp.tile([P, RB], f32)
            nc.scalar.activation(xsq[:], xl[:], Act.Square)
            sroot = sp.tile([P, RB], f32)
            nc.scalar.activation(sroot[:], xsq[:], Act.Sqrt, scale=-1.0, bias=1.0)

            # cos_m = xl*cos_g - s*sin_g
            ta = sp.tile([P, RB], f32)
            nc.vector.tensor_tensor(out=ta[:], in0=xl[:], in1=cos_g[:], op=Alu.mult)
            tb = sp.tile([P, RB], f32)
            nc.vector.tensor_tensor(out=tb[:], in0=sroot[:], in1=sin_g[:], op=Alu.mult)
            cosm = sp.tile([P, RB], f32)
            nc.vector.tensor_tensor(out=cosm[:], in0=ta[:], in1=tb[:], op=Alu.subtract)

            # lower-clip case: theta+g < eps  <=>  ms > -EPS/M  AND  xl > cos(eps-g)
            m1 = sp.tile([P, RB], f32)
            nc.vector.tensor_scalar(
                out=m1[:], in0=ms[:], scalar1=-EPS / M, scalar2=None, op0=Alu.is_gt
            )
            t1 = sp.tile([P, RB], f32)
            nc.vector.tensor_scalar(
                out=t1[:], in0=cos_g[:], scalar1=CE, scalar2=None, op0=Alu.mult
            )
            t2 = sp.tile([P, RB], f32)
            nc.vector.tensor_scalar(
                out=t2[:], in0=sin_g[:], scalar1=SE, scalar2=None, op0=Alu.mult
            )
            thresh = sp.tile([P, RB], f32)
            nc.vector.tensor_tensor(out=thresh[:], in0=t1[:], in1=t2[:], op=Alu.add)
            m2 = sp.tile([P, RB], f32)
            nc.vector.tensor_tensor(out=m2[:], in0=xl[:], in1=thresh[:], op=Alu.is_gt)
            maskc = sp.tile([P, RB], f32)
            nc.vector.tensor_tensor(out=maskc[:], in0=m1[:], in1=m2[:], op=Alu.mult)
            # cosm = cosm + mask * (CE - cosm)
            dce = sp.tile([P, RB], f32)
            nc.vector.tensor_scalar(
                out=dce[:], in0=cosm[:], scalar1=-1.0, scalar2=CE,
                op0=Alu.mult, op1=Alu.add,
            )
            mce = sp.tile([P, RB], f32)
            nc.vector.tensor_tensor(out=mce[:], in0=maskc[:], in1=dce[:], op=Alu.mult)
            nc.vector.tensor_tensor(out=cosm[:], in0=cosm[:], in1=mce[:], op=Alu.add)

            # fixv = S*(clip(cosm, -ce, ce) - M - M*ms)
            v = sp.tile([P, RB], f32)
            nc.vector.tensor_scalar(
                out=v[:], in0=cosm[:], scalar1=-CE, scalar2=CE,
                op0=Alu.max, op1=Alu.min,
            )
            q = sp.tile([P, RB], f32)
            nc.vector.tensor_scalar(
                out=q[:], in0=v[:], scalar1=S, scalar2=-S * M,
                op0=Alu.mult, op1=Alu.add,
            )
            r_ = sp.tile([P, RB], f32)
            nc.vector.tensor_scalar(
                out=r_[:], in0=ms[:], scalar1=S * M, scalar2=None, op0=Alu.mult
            )
            fixv = sp.tile([P, RB], f32)
            nc.vector.tensor_tensor(out=fixv[:], in0=q[:], in1=r_[:], op=Alu.subtract)

            # ---- streaming bulk pass: out = min(x, ce) * S ----
            for rb in range(RB):
                t = stp.tile([P, CS], f32)
                rows = slice(rb * P, (rb + 1) * P)
                nc.sync.dma_start(out=t[:], in_=cos_in.ap()[rows, :])
                nc.vector.tensor_scalar(
                    out=t[:], in0=t[:], scalar1=CE, scalar2=S,
                    op0=Alu.min, op1=Alu.mult,
                )
                nc.scalar.dma_start(out=out_t.ap()[rows, :], in_=t[:])

            # ---- scatter label fix-ups (Tile orders these after the stores) ----
            for j in range(k_cols):
                nc.gpsimd.indirect_dma_start(
                    out=out_t.ap(),
                    out_offset=bass.IndirectOffsetOnAxis(
                        ap=off[:, j : j + 1], axis=1
                    ),
                    in_=fixv[:, j : j + 1],
                    in_offset=None,
                    bounds_check=N * CS - 1,
                    oob_is_err=False,
                )

    nc.compile()
    return nc


def _get_compiled(k_cols):
    if k_cols not in _COMPILED:
        _COMPILED[k_cols] = _build(k_cols)
    return _COMPILED[k_cols]


def _make_in_maps(cosine, norms, label):
    """Shard cosine over C; build per-core [128, 16] tables of norms and
    flat gather/scatter offsets.  Rows are permuted into slots (p, j)
    (slot -> row mapping is free: batch stats are order-invariant) such
    that owned rows occupy the lowest slot columns; returns the number of
    columns k_cols the kernel must gather/scatter."""
    cos = np.ascontiguousarray(np.asarray(cosine, dtype=np.float32))
    nr = np.asarray(norms, dtype=np.float32).reshape(-1)
    lab = np.asarray(label).astype(np.int64).reshape(-1)
    assert cos.shape == (N, C) and nr.shape == (N,) and lab.shape == (N,)

    rows = np.arange(N, dtype=np.int64)
    in_maps = []
    max_owned = 0
    for i in range(NCORES):
        c0 = i * CS
        owned = (lab != -1) & (lab >= c0) & (lab < c0 + CS)
        n_owned = int(owned.sum())
        max_owned = max(max_owned, n_owned)
        # permutation: owned rows first, then the rest
        perm = np.concatenate([rows[owned], rows[~owned]])
        offv = np.where(
            owned[perm], perm * CS + (lab[perm] - c0), np.int64(SENTINEL)
        ).astype(np.int32)
        # slot (p, j) = permuted position j*128 + p  ->  table[p, j]
        off_tab = np.ascontiguousarray(offv.reshape(RB, P).T)
        norms_tab = np.ascontiguousarray(nr[perm].reshape(RB, P).T)
        in_maps.append(
            {
                "cosine": np.ascontiguousarray(cos[:, c0 : c0 + CS]),
                "norms_t": norms_tab,
                "off": off_tab,
            }
        )
    k_cols = max(2, -(-max_owned // P))
    return in_maps, k_cols


def _run(in_maps, k_cols, trace=False, **kwargs):
    import sys

    if "/opt/trn_rl_repo" not in sys.path:
        sys.path.insert(0, "/opt/trn_rl_repo")
    from concourse.bass_utils import run_bass_kernel_spmd

    nc = _get_compiled(k_cols)
    return run_bass_kernel_spmd(
        nc, in_maps, core_ids=list(range(NCORES)), trace=trace, **kwargs
    )


def kernel(cosine, norms, label):
    in_maps, k_cols = _make_in_maps(cosine, norms, label)
    res = _run(in_maps, k_cols)
    outs = [res.results[i]["out"] for i in range(NCORES)]
    return np.concatenate(outs, axis=1)


# revision 16
# speedup vs baseline: 1.2505x; 1.0993x over previous
"""AdaFace loss kernel for 8 TRN2 NeuronCores.

Math notes (reference is AdaFace with T_ALPHA=1, labels all valid):
  - Off-label columns: cos(clip(arccos(x), eps, pi-eps)) == min(x, cos(eps))
    exactly for x in [0, 1), so the [N, C] bulk is one dual-op
    tensor_scalar pass (min with cos(eps), then mult by S).
  - Label column per row: with theta = arccos(xl), g = -M*ms,
    cos(theta + g) = xl*cos(g) - sqrt(1-xl^2)*sin(g).  The lower clip
    (theta+g < eps -> eps) triggers iff eps-g > 0 AND xl > cos(eps-g);
    cos(eps-g) = ce*cos(g) + se*sin(g).  Upper clip can't trigger.
    Final label value: S * (clip(cos_m, -ce, ce) - (M + M*ms)).
  - Sharding: C split across 8 cores (6250 cols each); norms/labels are
    tiny and replicated so batch stats are computed redundantly per core
    (no collectives).  Label fix-ups applied with indirect DMA
    gather/scatter using flat offsets; rows whose label falls outside a
    core's shard get a huge sentinel offset and are skipped via the
    bounds check.
"""

import math

import numpy as np

N = 2048
C = 50000
NCORES = 8
CS = C // NCORES  # 6250 columns per core
P = 128
RB = N // P  # 16 row blocks

M = 0.4
H = 0.333
S = 64.0
EPS = 1e-3

CE = float(np.cos(np.float32(EPS), dtype=np.float32))  # cos(eps) in f32
SE = float(np.sin(np.float32(EPS), dtype=np.float32))  # sin(eps) in f32
SENTINEL = np.int32(1 << 30)

_COMPILED = {}


def _build(k_cols):
    import sys

    if "/opt/trn_rl_repo" not in sys.path:
        sys.path.insert(0, "/opt/trn_rl_repo")

    import concourse.bass as bass
    import concourse.tile as tile
    from concourse import bacc, bass_isa, mybir

    f32 = mybir.dt.float32
    bf16 = mybir.dt.bfloat16
    i32 = mybir.dt.int32
    Alu = mybir.AluOpType
    Act = mybir.ActivationFunctionType

    nc = bacc.Bacc(
        "TRN2",
        target_bir_lowering=False,
        debug=False,
        enable_asserts=False,
        num_devices=NCORES,
    )

    cos_in = nc.dram_tensor("cosine", [N, CS], f32, kind="ExternalInput")
    norms_t = nc.dram_tensor("norms_t", [P, RB], f32, kind="ExternalInput")
    off_t = nc.dram_tensor("off", [P, RB], i32, kind="ExternalInput")
    out_t = nc.dram_tensor("out", [N, CS], bf16, kind="ExternalOutput")

    with tile.TileContext(nc) as tc:
        with (
            tc.tile_pool(name="small", bufs=1) as sp,
            tc.tile_pool(name="stream", bufs=4) as stp,
        ):
            # ---- per-row margin scalars (all [P, RB]; row r = rb*128 + p) ----
            nt = sp.tile([P, RB], f32)
            nc.sync.dma_start(out=nt[:], in_=norms_t.ap())
            off = sp.tile([P, RB], i32)
            nc.sync.dma_start(out=off[:], in_=off_t.ap())

            n_c = sp.tile([P, RB], f32)
            nc.vector.tensor_scalar(
                out=n_c[:], in0=nt[:], scalar1=EPS, scalar2=100.0,
                op0=Alu.max, op1=Alu.min,
            )

            # mean over all 2048 rows (partition all-reduce + free reduce)
            ar1 = sp.tile([P, RB], f32)
            nc.gpsimd.partition_all_reduce(ar1[:], n_c[:], P, bass_isa.ReduceOp.add)
            mean = sp.tile([P, 1], f32)
            nc.vector.tensor_reduce(
                out=mean[:], in_=ar1[:], axis=mybir.AxisListType.X, op=Alu.add
            )
            nc.vector.tensor_scalar(
                out=mean[:], in0=mean[:], scalar1=1.0 / N, scalar2=None, op0=Alu.mult
            )

            diff = sp.tile([P, RB], f32)
            nc.vector.tensor_scalar(
                out=diff[:], in0=n_c[:], scalar1=mean[:, :1], scalar2=None,
                op0=Alu.subtract,
            )
            sq = sp.tile([P, RB], f32)
            nc.vector.tensor_tensor(out=sq[:], in0=diff[:], in1=diff[:], op=Alu.mult)
            ar2 = sp.tile([P, RB], f32)
            nc.gpsimd.partition_all_reduce(ar2[:], sq[:], P, bass_isa.ReduceOp.add)
            var = sp.tile([P, 1], f32)
            nc.vector.tensor_reduce(
                out=var[:], in_=ar2[:], axis=mybir.AxisListType.X, op=Alu.add
            )
            nc.vector.tensor_scalar(
                out=var[:], in0=var[:], scalar1=1.0 / (N - 1), scalar2=None,
                op0=Alu.mult,
            )
            std = sp.tile([P, 1], f32)
            nc.scalar.activation(std[:], var[:], Act.Sqrt)
            nc.vector.tensor_scalar(
                out=std[:], in0=std[:], scalar1=EPS, scalar2=None, op0=Alu.add
            )
            dinv = sp.tile([P, 1], f32)
            nc.vector.reciprocal(dinv[:], std[:])

            ms = sp.tile([P, RB], f32)
            nc.vector.tensor_scalar(
                out=ms[:], in0=diff[:], scalar1=dinv[:, :1], scalar2=H,
                op0=Alu.mult, op1=Alu.mult,
            )
            nc.vector.tensor_scalar(
                out=ms[:], in0=ms[:], scalar1=-1.0, scalar2=1.0,
                op0=Alu.max, op1=Alu.min,
            )

            # g = -M*ms;  sin(g), cos(g) via the Sin LUT (cos is even)
            halfpi = sp.tile([P, 1], f32)
            nc.vector.memset(halfpi[:], math.pi / 2)
            sin_g = sp.tile([P, RB], f32)
            nc.scalar.activation(sin_g[:], ms[:], Act.Sin, scale=-M)
            cos_g = sp.tile([P, RB], f32)
            nc.scalar.activation(
                cos_g[:], ms[:], Act.Sin, scale=-M, bias=halfpi[:, :1]
            )

            # ---- gather xl = cosine[r, label[r]] (local shard only) ----
            # One call per rb column: the HW DGE emits one descriptor per
            # partition, moving that partition's whole free-dim run and
            # consuming one offset per partition — so gather/scatter must
            # use [128, 1] data+offset slices to move exactly one element
            # per row.
            # Rows are slot-permuted on the host so all rows whose label
            # falls in this core's shard live in the first k_cols columns
            # — only those columns need gather/scatter calls.
            xl = sp.tile([P, RB], f32)
            nc.vector.memset(xl[:], 0.0)
            for j in range(k_cols):
                nc.gpsimd.indirect_dma_start(
                    out=xl[:, j : j + 1],
                    out_offset=None,
                    in_=cos_in.ap(),
                    in_offset=bass.IndirectOffsetOnAxis(
                        ap=off[:, j : j + 1], axis=1
                    ),
                    bounds_check=N * CS - 1,
                    oob_is_err=False,
                )

            # s = sqrt(1 - xl^2)
            xsq = sp.tile([P, RB], f32)
            nc.scalar.activation(xsq[:], xl[:], Act.Square)
            sroot = sp.tile([P, RB], f32)
            nc.scalar.activation(sroot[:], xsq[:], Act.Sqrt, scale=-1.0, bias=1.0)

            # cos_m = xl*cos_g - s*sin_g
            ta = sp.tile([P, RB], f32)
            nc.vector.tensor_tensor(out=ta[:], in0=xl[:], in1=cos_g[:], op=Alu.mult)
            tb = sp.tile([P, RB], f32)
            nc.vector.tensor_tensor(out=tb[:], in0=sroot[:], in1=sin_g[:], op=Alu.mult)
            cosm = sp.tile([P, RB], f32)
            nc.vector.tensor_tensor(out=cosm[:], in0=ta[:], in1=tb[:], op=Alu.subtract)

            # lower-clip case: theta+g < eps  <=>  ms > -EPS/M  AND  xl > cos(eps-g)
            m1 = sp.tile([P, RB], f32)
            nc.vector.tensor_scalar(
                out=m1[:], in0=ms[:], scalar1=-EPS / M, scalar2=None, op0=Alu.is_gt
            )
            t1 = sp.tile([P, RB], f32)
            nc.vector.tensor_scalar(
                out=t1[:], in0=cos_g[:], scalar1=CE, scalar2=None, op0=Alu.mult
            )
            t2 = sp.tile([P, RB], f32)
            nc.vector.tensor_scalar(
                out=t2[:], in0=sin_g[:], scalar1=SE, scalar2=None, op0=Alu.mult
            )
            thresh = sp.tile([P, RB], f32)
            nc.vector.tensor_tensor(out=thresh[:], in0=t1[:], in1=t2[:], op=Alu.add)
            m2 = sp.tile([P, RB], f32)
            nc.vector.tensor_tensor(out=m2[:], in0=xl[:], in1=thresh[:], op=Alu.is_gt)
            maskc = sp.tile([P, RB], f32)
            nc.vector.tensor_tensor(out=maskc[:], in0=m1[:], in1=m2[:], op=Alu.mult)
            # cosm = cosm + mask * (CE - cosm)
            dce = sp.tile([P, RB], f32)
            nc.vector.tensor_scalar(
                out=dce[:], in0=cosm[:], scalar1=-1.0, scalar2=CE,
                op0=Alu.mult, op1=Alu.add,
            )
            mce = sp.tile([P, RB], f32)
            nc.vector.tensor_tensor(out=mce[:], in0=maskc[:], in1=dce[:], op=Alu.mult)
            nc.vector.tensor_tensor(out=cosm[:], in0=cosm[:], in1=mce[:], op=Alu.add)

            # fixv = S*(clip(cosm, -ce, ce) - M - M*ms)
            v = sp.tile([P, RB], f32)
            nc.vector.tensor_scalar(
                out=v[:], in0=cosm[:], scalar1=-CE, scalar2=CE,
                op0=Alu.max, op1=Alu.min,
            )
            q = sp.tile([P, RB], f32)
            nc.vector.tensor_scalar(
                out=q[:], in0=v[:], scalar1=S, scalar2=-S * M,
                op0=Alu.mult, op1=Alu.add,
            )
            r_ = sp.tile([P, RB], f32)
            nc.vector.tensor_scalar(
                out=r_[:], in0=ms[:], scalar1=S * M, scalar2=None, op0=Alu.mult
            )
            fixv = sp.tile([P, RB], bf16)
            nc.vector.tensor_tensor(out=fixv[:], in0=q[:], in1=r_[:], op=Alu.subtract)

            # ---- streaming bulk pass: out = min(x, ce) * S (bf16 stores) ----
            for rb in range(RB):
                t = stp.tile([P, CS], f32)
                rows = slice(rb * P, (rb + 1) * P)
                nc.sync.dma_start(out=t[:], in_=cos_in.ap()[rows, :])
                tb = stp.tile([P, CS], bf16)
                nc.vector.tensor_scalar(
                    out=tb[:], in0=t[:], scalar1=CE, scalar2=S,
                    op0=Alu.min, op1=Alu.mult,
                )
                nc.scalar.dma_start(out=out_t.ap()[rows, :], in_=tb[:])

            # ---- scatter label fix-ups (Tile orders these after the stores) ----
            for j in range(k_cols):
                nc.gpsimd.indirect_dma_start(
                    out=out_t.ap(),
                    out_offset=bass.IndirectOffsetOnAxis(
                        ap=off[:, j : j + 1], axis=1
                    ),
                    in_=fixv[:, j : j + 1],
                    in_offset=None,
                    bounds_check=N * CS - 1,
                    oob_is_err=False,
                )

    nc.compile()
    return nc


def _get_compiled(k_cols):
    if k_cols not in _COMPILED:
        _COMPILED[k_cols] = _build(k_cols)
    return _COMPILED[k_cols]


def _make_in_maps(cosine, norms, label):
    """Shard cosine over C; build per-core [128, 16] tables of norms and
    flat gather/scatter offsets.  Rows are permuted into slots (p, j)
    (slot -> row mapping is free: batch stats are order-invariant) such
    that owned rows occupy the lowest slot columns; returns the number of
    columns k_cols the kernel must gather/scatter."""
    cos = np.ascontiguousarray(np.asarray(cosine, dtype=np.float32))
    nr = np.asarray(norms, dtype=np.float32).reshape(-1)
    lab = np.asarray(label).astype(np.int64).reshape(-1)
    assert cos.shape == (N, C) and nr.shape == (N,) and lab.shape == (N,)

    rows = np.arange(N, dtype=np.int64)
    in_maps = []
    max_owned = 0
    for i in range(NCORES):
        c0 = i * CS
        owned = (lab != -1) & (lab >= c0) & (lab < c0 + CS)
        n_owned = int(owned.sum())
        max_owned = max(max_owned, n_owned)
        # permutation: owned rows first, then the rest
        perm = np.concatenate([rows[owned], rows[~owned]])
        offv = np.where(
            owned[perm], perm * CS + (lab[perm] - c0), np.int64(SENTINEL)
        ).astype(np.int32)
        # slot (p, j) = permuted position j*128 + p  ->  table[p, j]
        off_tab = np.ascontiguousarray(offv.reshape(RB, P).T)
        norms_tab = np.ascontiguousarray(nr[perm].reshape(RB, P).T)
        in_maps.append(
            {
                "cosine": np.ascontiguousarray(cos[:, c0 : c0 + CS]),
                "norms_t": norms_tab,
                "off": off_tab,
            }
        )
    k_cols = max(2, -(-max_owned // P))
    return in_maps, k_cols


def _run(in_maps, k_cols, trace=False, **kwargs):
    import sys

    if "/opt/trn_rl_repo" not in sys.path:
        sys.path.insert(0, "/opt/trn_rl_repo")
    from concourse.bass_utils import run_bass_kernel_spmd

    nc = _get_compiled(k_cols)
    return run_bass_kernel_spmd(
        nc, in_maps, core_ids=list(range(NCORES)), trace=trace, **kwargs
    )


def kernel(cosine, norms, label):
    in_maps, k_cols = _make_in_maps(cosine, norms, label)
    res = _run(in_maps, k_cols)
    outs = [np.asarray(res.results[i]["out"]) for i in range(NCORES)]
    return np.concatenate(outs, axis=1).astype(np.float32)


# revision 20
# speedup vs baseline: 1.8116x; 1.4487x over previous
"""AdaFace loss kernel for 8 TRN2 NeuronCores.

Math notes (reference is AdaFace with T_ALPHA=1, labels all valid):
  - Off-label columns: cos(clip(arccos(x), eps, pi-eps)) == min(x, cos(eps))
    exactly for x in [0, 1), so the [N, C] bulk is one dual-op
    tensor_scalar pass (min with cos(eps), then mult by S).
  - Label column per row: with theta = arccos(xl), g = -M*ms,
    cos(theta + g) = xl*cos(g) - sqrt(1-xl^2)*sin(g).  The lower clip
    (theta+g < eps -> eps) triggers iff eps-g > 0 AND xl > cos(eps-g);
    cos(eps-g) = ce*cos(g) + se*sin(g).  Upper clip can't trigger.
    Final label value: S * (clip(cos_m, -ce, ce) - (M + M*ms)).
  - Sharding: C split across 8 cores (6250 cols each); norms/labels are
    tiny and replicated so batch stats are computed redundantly per core
    (no collectives).  Label fix-ups applied with indirect DMA
    gather/scatter using flat offsets; rows whose label falls outside a
    core's shard get a huge sentinel offset and are skipped via the
    bounds check.
"""

import math

import ml_dtypes
import numpy as np

N = 2048
C = 50000
NCORES = 8
CS = C // NCORES  # 6250 columns per core
P = 128
RB = N // P  # 16 row blocks

M = 0.4
H = 0.333
S = 64.0
EPS = 1e-3

CE = float(np.cos(np.float32(EPS), dtype=np.float32))  # cos(eps) in f32
SE = float(np.sin(np.float32(EPS), dtype=np.float32))  # sin(eps) in f32
SENTINEL = np.int32(1 << 30)

_COMPILED = {}


def _build(k_cols):
    import sys

    if "/opt/trn_rl_repo" not in sys.path:
        sys.path.insert(0, "/opt/trn_rl_repo")

    import concourse.bass as bass
    import concourse.tile as tile
    from concourse import bacc, bass_isa, mybir

    f32 = mybir.dt.float32
    bf16 = mybir.dt.bfloat16
    i32 = mybir.dt.int32
    Alu = mybir.AluOpType
    Act = mybir.ActivationFunctionType

    nc = bacc.Bacc(
        "TRN2",
        target_bir_lowering=False,
        debug=False,
        enable_asserts=False,
        num_devices=NCORES,
    )

    cos_in = nc.dram_tensor("cosine", [N, CS], f32, kind="ExternalInput")
    cos_bf = nc.dram_tensor("cosine_bf", [N, CS], bf16, kind="ExternalInput")
    norms_t = nc.dram_tensor("norms_t", [P, RB], f32, kind="ExternalInput")
    off_t = nc.dram_tensor("off", [P, RB], i32, kind="ExternalInput")
    out_t = nc.dram_tensor("out", [N, CS], bf16, kind="ExternalOutput")

    with tile.TileContext(nc) as tc:
        with (
            tc.tile_pool(name="small", bufs=1) as sp,
            tc.tile_pool(name="stream", bufs=4) as stp,
        ):
            # ---- per-row margin scalars (all [P, RB]; row r = rb*128 + p) ----
            nt = sp.tile([P, RB], f32)
            nc.sync.dma_start(out=nt[:], in_=norms_t.ap())
            off = sp.tile([P, RB], i32)
            nc.sync.dma_start(out=off[:], in_=off_t.ap())

            n_c = sp.tile([P, RB], f32)
            nc.vector.tensor_scalar(
                out=n_c[:], in0=nt[:], scalar1=EPS, scalar2=100.0,
                op0=Alu.max, op1=Alu.min,
            )

            # mean over all 2048 rows (partition all-reduce + free reduce)
            ar1 = sp.tile([P, RB], f32)
            nc.gpsimd.partition_all_reduce(ar1[:], n_c[:], P, bass_isa.ReduceOp.add)
            mean = sp.tile([P, 1], f32)
            nc.vector.tensor_reduce(
                out=mean[:], in_=ar1[:], axis=mybir.AxisListType.X, op=Alu.add
            )
            nc.vector.tensor_scalar(
                out=mean[:], in0=mean[:], scalar1=1.0 / N, scalar2=None, op0=Alu.mult
            )

            diff = sp.tile([P, RB], f32)
            nc.vector.tensor_scalar(
                out=diff[:], in0=n_c[:], scalar1=mean[:, :1], scalar2=None,
                op0=Alu.subtract,
            )
            sq = sp.tile([P, RB], f32)
            nc.vector.tensor_tensor(out=sq[:], in0=diff[:], in1=diff[:], op=Alu.mult)
            ar2 = sp.tile([P, RB], f32)
            nc.gpsimd.partition_all_reduce(ar2[:], sq[:], P, bass_isa.ReduceOp.add)
            var = sp.tile([P, 1], f32)
            nc.vector.tensor_reduce(
                out=var[:], in_=ar2[:], axis=mybir.AxisListType.X, op=Alu.add
            )
            nc.vector.tensor_scalar(
                out=var[:], in0=var[:], scalar1=1.0 / (N - 1), scalar2=None,
                op0=Alu.mult,
            )
            std = sp.tile([P, 1], f32)
            nc.scalar.activation(std[:], var[:], Act.Sqrt)
            nc.vector.tensor_scalar(
                out=std[:], in0=std[:], scalar1=EPS, scalar2=None, op0=Alu.add
            )
            dinv = sp.tile([P, 1], f32)
            nc.vector.reciprocal(dinv[:], std[:])

            ms = sp.tile([P, RB], f32)
            nc.vector.tensor_scalar(
                out=ms[:], in0=diff[:], scalar1=dinv[:, :1], scalar2=H,
                op0=Alu.mult, op1=Alu.mult,
            )
            nc.vector.tensor_scalar(
                out=ms[:], in0=ms[:], scalar1=-1.0, scalar2=1.0,
                op0=Alu.max, op1=Alu.min,
            )

            # g = -M*ms;  sin(g), cos(g) via the Sin LUT (cos is even)
            halfpi = sp.tile([P, 1], f32)
            nc.vector.memset(halfpi[:], math.pi / 2)
            sin_g = sp.tile([P, RB], f32)
            nc.scalar.activation(sin_g[:], ms[:], Act.Sin, scale=-M)
            cos_g = sp.tile([P, RB], f32)
            nc.scalar.activation(
                cos_g[:], ms[:], Act.Sin, scale=-M, bias=halfpi[:, :1]
            )

            # ---- gather xl = cosine[r, label[r]] (local shard only) ----
            # One call per rb column: the HW DGE emits one descriptor per
            # partition, moving that partition's whole free-dim run and
            # consuming one offset per partition — so gather/scatter must
            # use [128, 1] data+offset slices to move exactly one element
            # per row.
            # Rows are slot-permuted on the host so all rows whose label
            # falls in this core's shard live in the first k_cols columns
            # — only those columns need gather/scatter calls.
            xl = sp.tile([P, RB], f32)
            nc.vector.memset(xl[:], 0.0)
            for j in range(k_cols):
                nc.gpsimd.indirect_dma_start(
                    out=xl[:, j : j + 1],
                    out_offset=None,
                    in_=cos_in.ap(),
                    in_offset=bass.IndirectOffsetOnAxis(
                        ap=off[:, j : j + 1], axis=1
                    ),
                    bounds_check=N * CS - 1,
                    oob_is_err=False,
                )

            # s = sqrt(1 - xl^2)
            xsq = sp.tile([P, RB], f32)
            nc.scalar.activation(xsq[:], xl[:], Act.Square)
            sroot = sp.tile([P, RB], f32)
            nc.scalar.activation(sroot[:], xsq[:], Act.Sqrt, scale=-1.0, bias=1.0)

            # cos_m = xl*cos_g - s*sin_g
            ta = sp.tile([P, RB], f32)
            nc.vector.tensor_tensor(out=ta[:], in0=xl[:], in1=cos_g[:], op=Alu.mult)
            tb = sp.tile([P, RB], f32)
            nc.vector.tensor_tensor(out=tb[:], in0=sroot[:], in1=sin_g[:], op=Alu.mult)
            cosm = sp.tile([P, RB], f32)
            nc.vector.tensor_tensor(out=cosm[:], in0=ta[:], in1=tb[:], op=Alu.subtract)

            # lower-clip case: theta+g < eps  <=>  ms > -EPS/M  AND  xl > cos(eps-g)
            m1 = sp.tile([P, RB], f32)
            nc.vector.tensor_scalar(
                out=m1[:], in0=ms[:], scalar1=-EPS / M, scalar2=None, op0=Alu.is_gt
            )
            t1 = sp.tile([P, RB], f32)
            nc.vector.tensor_scalar(
                out=t1[:], in0=cos_g[:], scalar1=CE, scalar2=None, op0=Alu.mult
            )
            t2 = sp.tile([P, RB], f32)
            nc.vector.tensor_scalar(
                out=t2[:], in0=sin_g[:], scalar1=SE, scalar2=None, op0=Alu.mult
            )
            thresh = sp.tile([P, RB], f32)
            nc.vector.tensor_tensor(out=thresh[:], in0=t1[:], in1=t2[:], op=Alu.add)
            m2 = sp.tile([P, RB], f32)
            nc.vector.tensor_tensor(out=m2[:], in0=xl[:], in1=thresh[:], op=Alu.is_gt)
            maskc = sp.tile([P, RB], f32)
            nc.vector.tensor_tensor(out=maskc[:], in0=m1[:], in1=m2[:], op=Alu.mult)
            # cosm = cosm + mask * (CE - cosm)
            dce = sp.tile([P, RB], f32)
            nc.vector.tensor_scalar(
                out=dce[:], in0=cosm[:], scalar1=-1.0, scalar2=CE,
                op0=Alu.mult, op1=Alu.add,
            )
            mce = sp.tile([P, RB], f32)
            nc.vector.tensor_tensor(out=mce[:], in0=maskc[:], in1=dce[:], op=Alu.mult)
            nc.vector.tensor_tensor(out=cosm[:], in0=cosm[:], in1=mce[:], op=Alu.add)

            # fixv = S*(clip(cosm, -ce, ce) - M - M*ms)
            v = sp.tile([P, RB], f32)
            nc.vector.tensor_scalar(
                out=v[:], in0=cosm[:], scalar1=-CE, scalar2=CE,
                op0=Alu.max, op1=Alu.min,
            )
            q = sp.tile([P, RB], f32)
            nc.vector.tensor_scalar(
                out=q[:], in0=v[:], scalar1=S, scalar2=-S * M,
                op0=Alu.mult, op1=Alu.add,
            )
            r_ = sp.tile([P, RB], f32)
            nc.vector.tensor_scalar(
                out=r_[:], in0=ms[:], scalar1=S * M, scalar2=None, op0=Alu.mult
            )
            fixv = sp.tile([P, RB], bf16)
            nc.vector.tensor_tensor(out=fixv[:], in0=q[:], in1=r_[:], op=Alu.subtract)

            # ---- streaming bulk pass: out = min(x, ce) * S  (bf16 in/out;
            # the DVE computes in fp32 internally with f32 immediates) ----
            for rb in range(RB):
                t = stp.tile([P, CS], bf16)
                rows = slice(rb * P, (rb + 1) * P)
                nc.sync.dma_start(out=t[:], in_=cos_bf.ap()[rows, :])
                nc.vector.tensor_scalar(
                    out=t[:], in0=t[:], scalar1=CE, scalar2=S,
                    op0=Alu.min, op1=Alu.mult,
                )
                nc.scalar.dma_start(out=out_t.ap()[rows, :], in_=t[:])

            # ---- scatter label fix-ups (Tile orders these after the stores) ----
            for j in range(k_cols):
                nc.gpsimd.indirect_dma_start(
                    out=out_t.ap(),
                    out_offset=bass.IndirectOffsetOnAxis(
                        ap=off[:, j : j + 1], axis=1
                    ),
                    in_=fixv[:, j : j + 1],
                    in_offset=None,
                    bounds_check=N * CS - 1,
                    oob_is_err=False,
                )

    nc.compile()
    return nc


def _get_compiled(k_cols):
    if k_cols not in _COMPILED:
        _COMPILED[k_cols] = _build(k_cols)
    return _COMPILED[k_cols]


def _make_in_maps(cosine, norms, label):
    """Shard cosine over C; build per-core [128, 16] tables of norms and
    flat gather/scatter offsets.  Rows are permuted into slots (p, j)
    (slot -> row mapping is free: batch stats are order-invariant) such
    that owned rows occupy the lowest slot columns; returns the number of
    columns k_cols the kernel must gather/scatter."""
    cos = np.ascontiguousarray(np.asarray(cosine, dtype=np.float32))
    nr = np.asarray(norms, dtype=np.float32).reshape(-1)
    lab = np.asarray(label).astype(np.int64).reshape(-1)
    assert cos.shape == (N, C) and nr.shape == (N,) and lab.shape == (N,)

    rows = np.arange(N, dtype=np.int64)
    in_maps = []
    max_owned = 0
    for i in range(NCORES):
        c0 = i * CS
        owned = (lab != -1) & (lab >= c0) & (lab < c0 + CS)
        n_owned = int(owned.sum())
        max_owned = max(max_owned, n_owned)
        # permutation: owned rows first, then the rest
        perm = np.concatenate([rows[owned], rows[~owned]])
        offv = np.where(
            owned[perm], perm * CS + (lab[perm] - c0), np.int64(SENTINEL)
        ).astype(np.int32)
        # slot (p, j) = permuted position j*128 + p  ->  table[p, j]
        off_tab = np.ascontiguousarray(offv.reshape(RB, P).T)
        norms_tab = np.ascontiguousarray(nr[perm].reshape(RB, P).T)
        cos_slice = np.ascontiguousarray(cos[:, c0 : c0 + CS])
        in_maps.append(
            {
                "cosine": cos_slice,
                "cosine_bf": cos_slice.astype(ml_dtypes.bfloat16),
                "norms_t": norms_tab,
                "off": off_tab,
            }
        )
    k_cols = max(2, -(-max_owned // P))
    return in_maps, k_cols


def _run(in_maps, k_cols, trace=False, **kwargs):
    import sys

    if "/opt/trn_rl_repo" not in sys.path:
        sys.path.insert(0, "/opt/trn_rl_repo")
    from concourse.bass_utils import run_bass_kernel_spmd

    nc = _get_compiled(k_cols)
    return run_bass_kernel_spmd(
        nc, in_maps, core_ids=list(range(NCORES)), trace=trace, **kwargs
    )


def kernel(cosine, norms, label):
    in_maps, k_cols = _make_in_maps(cosine, norms, label)
    res = _run(in_maps, k_cols)
    outs = [np.asarray(res.results[i]["out"]) for i in range(NCORES)]
    return np.concatenate(outs, axis=1).astype(np.float32)


# revision 28
# speedup vs baseline: 2.1478x; 1.1856x over previous
"""AdaFace loss kernel for 8 TRN2 NeuronCores.

Math notes (reference is AdaFace with T_ALPHA=1, labels all valid):
  - Off-label columns: cos(clip(arccos(x), eps, pi-eps)) == min(x, cos(eps))
    exactly for x in [0, 1), so the [N, C] bulk is one dual-op
    tensor_scalar pass (min with cos(eps), then mult by S).
  - Label column per row: with theta = arccos(xl), g = -M*ms,
    cos(theta + g) = xl*cos(g) - sqrt(1-xl^2)*sin(g).  The lower clip
    (theta+g < eps -> eps) triggers iff eps-g > 0 AND xl > cos(eps-g);
    cos(eps-g) = ce*cos(g) + se*sin(g).  Upper clip can't trigger.
    Final label value: S * (clip(cos_m, -ce, ce) - (M + M*ms)).
  - Sharding: C split across 8 cores (6250 cols each); norms/labels are
    tiny and replicated so batch stats are computed redundantly per core
    (no collectives).  Label fix-ups applied with indirect DMA
    gather/scatter using flat offsets; rows whose label falls outside a
    core's shard get a huge sentinel offset and are skipped via the
    bounds check.
"""

import math

import ml_dtypes
import numpy as np

N = 2048
C = 50000
NCORES = 8
CS = C // NCORES  # 6250 columns per core
P = 128
RB = N // P  # 16 row blocks

M = 0.4
H = 0.333
S = 64.0
EPS = 1e-3

CE = float(np.cos(np.float32(EPS), dtype=np.float32))  # cos(eps) in f32
SE = float(np.sin(np.float32(EPS), dtype=np.float32))  # sin(eps) in f32
SENTINEL = np.int32(1 << 30)

_COMPILED = {}


def _build(k_cols, blocks):
    import sys

    if "/opt/trn_rl_repo" not in sys.path:
        sys.path.insert(0, "/opt/trn_rl_repo")

    import concourse.bass as bass
    import concourse.tile as tile
    from concourse import bacc, bass_isa, mybir

    f32 = mybir.dt.float32
    bf16 = mybir.dt.bfloat16
    i32 = mybir.dt.int32
    Alu = mybir.AluOpType
    Act = mybir.ActivationFunctionType

    nc = bacc.Bacc(
        "TRN2",
        target_bir_lowering=False,
        debug=False,
        enable_asserts=False,
        num_devices=NCORES,
    )

    cos_in = nc.dram_tensor("cosine", [N, CS], f32, kind="ExternalInput")
    cos_bf = nc.dram_tensor("cosine_bf", [N, CS], bf16, kind="ExternalInput")
    norms_t = nc.dram_tensor("norms_t", [P, RB], f32, kind="ExternalInput")
    off_t = nc.dram_tensor("off", [P, RB], i32, kind="ExternalInput")
    out_t = nc.dram_tensor("out", [N, CS], bf16, kind="ExternalOutput")

    with tile.TileContext(nc) as tc:
        with (
            tc.tile_pool(name="small", bufs=1) as sp,
            tc.tile_pool(name="stream", bufs=6) as stp,
        ):
            # ---- per-row margin scalars (all [P, RB]; row r = rb*128 + p) ----
            nt = sp.tile([P, RB], f32)
            nc.sync.dma_start(out=nt[:], in_=norms_t.ap())
            off = sp.tile([P, RB], i32)
            nc.sync.dma_start(out=off[:], in_=off_t.ap())

            n_c = sp.tile([P, RB], f32)
            nc.vector.tensor_scalar(
                out=n_c[:], in0=nt[:], scalar1=EPS, scalar2=100.0,
                op0=Alu.max, op1=Alu.min,
            )

            # mean over all 2048 rows (partition all-reduce + free reduce)
            ar1 = sp.tile([P, RB], f32)
            nc.gpsimd.partition_all_reduce(ar1[:], n_c[:], P, bass_isa.ReduceOp.add)
            mean = sp.tile([P, 1], f32)
            nc.vector.tensor_reduce(
                out=mean[:], in_=ar1[:], axis=mybir.AxisListType.X, op=Alu.add
            )
            nc.vector.tensor_scalar(
                out=mean[:], in0=mean[:], scalar1=1.0 / N, scalar2=None, op0=Alu.mult
            )

            diff = sp.tile([P, RB], f32)
            nc.vector.tensor_scalar(
                out=diff[:], in0=n_c[:], scalar1=mean[:, :1], scalar2=None,
                op0=Alu.subtract,
            )
            sq = sp.tile([P, RB], f32)
            nc.vector.tensor_tensor(out=sq[:], in0=diff[:], in1=diff[:], op=Alu.mult)
            ar2 = sp.tile([P, RB], f32)
            nc.gpsimd.partition_all_reduce(ar2[:], sq[:], P, bass_isa.ReduceOp.add)
            var = sp.tile([P, 1], f32)
            nc.vector.tensor_reduce(
                out=var[:], in_=ar2[:], axis=mybir.AxisListType.X, op=Alu.add
            )
            nc.vector.tensor_scalar(
                out=var[:], in0=var[:], scalar1=1.0 / (N - 1), scalar2=None,
                op0=Alu.mult,
            )
            std = sp.tile([P, 1], f32)
            nc.scalar.activation(std[:], var[:], Act.Sqrt)
            nc.vector.tensor_scalar(
                out=std[:], in0=std[:], scalar1=EPS, scalar2=None, op0=Alu.add
            )
            dinv = sp.tile([P, 1], f32)
            nc.vector.reciprocal(dinv[:], std[:])

            ms = sp.tile([P, RB], f32)
            nc.vector.tensor_scalar(
                out=ms[:], in0=diff[:], scalar1=dinv[:, :1], scalar2=H,
                op0=Alu.mult, op1=Alu.mult,
            )
            nc.vector.tensor_scalar(
                out=ms[:], in0=ms[:], scalar1=-1.0, scalar2=1.0,
                op0=Alu.max, op1=Alu.min,
            )

            # g = -M*ms;  sin(g), cos(g) via the Sin LUT (cos is even)
            halfpi = sp.tile([P, 1], f32)
            nc.vector.memset(halfpi[:], math.pi / 2)
            sin_g = sp.tile([P, RB], f32)
            nc.scalar.activation(sin_g[:], ms[:], Act.Sin, scale=-M)
            cos_g = sp.tile([P, RB], f32)
            nc.scalar.activation(
                cos_g[:], ms[:], Act.Sin, scale=-M, bias=halfpi[:, :1]
            )

            # ---- gather xl = cosine[r, label[r]] (local shard only) ----
            # One call per rb column: the HW DGE emits one descriptor per
            # partition, moving that partition's whole free-dim run and
            # consuming one offset per partition — so gather/scatter must
            # use [128, 1] data+offset slices to move exactly one element
            # per row.
            # Rows are slot-permuted on the host so all rows whose label
            # falls in this core's shard live in the first k_cols columns
            # — only those columns need gather/scatter calls.
            xl = sp.tile([P, RB], f32)
            nc.vector.memset(xl[:], 0.0)
            for j in range(k_cols):
                nc.gpsimd.indirect_dma_start(
                    out=xl[:, j : j + 1],
                    out_offset=None,
                    in_=cos_in.ap(),
                    in_offset=bass.IndirectOffsetOnAxis(
                        ap=off[:, j : j + 1], axis=1
                    ),
                    bounds_check=N * CS - 1,
                    oob_is_err=False,
                )

            # s = sqrt(1 - xl^2)
            xsq = sp.tile([P, RB], f32)
            nc.scalar.activation(xsq[:], xl[:], Act.Square)
            sroot = sp.tile([P, RB], f32)
            nc.scalar.activation(sroot[:], xsq[:], Act.Sqrt, scale=-1.0, bias=1.0)

            # cos_m = xl*cos_g - s*sin_g
            ta = sp.tile([P, RB], f32)
            nc.vector.tensor_tensor(out=ta[:], in0=xl[:], in1=cos_g[:], op=Alu.mult)
            tb = sp.tile([P, RB], f32)
            nc.vector.tensor_tensor(out=tb[:], in0=sroot[:], in1=sin_g[:], op=Alu.mult)
            cosm = sp.tile([P, RB], f32)
            nc.vector.tensor_tensor(out=cosm[:], in0=ta[:], in1=tb[:], op=Alu.subtract)

            # lower-clip case: theta+g < eps  <=>  ms > -EPS/M  AND  xl > cos(eps-g)
            m1 = sp.tile([P, RB], f32)
            nc.vector.tensor_scalar(
                out=m1[:], in0=ms[:], scalar1=-EPS / M, scalar2=None, op0=Alu.is_gt
            )
            t1 = sp.tile([P, RB], f32)
            nc.vector.tensor_scalar(
                out=t1[:], in0=cos_g[:], scalar1=CE, scalar2=None, op0=Alu.mult
            )
            t2 = sp.tile([P, RB], f32)
            nc.vector.tensor_scalar(
                out=t2[:], in0=sin_g[:], scalar1=SE, scalar2=None, op0=Alu.mult
            )
            thresh = sp.tile([P, RB], f32)
            nc.vector.tensor_tensor(out=thresh[:], in0=t1[:], in1=t2[:], op=Alu.add)
            m2 = sp.tile([P, RB], f32)
            nc.vector.tensor_tensor(out=m2[:], in0=xl[:], in1=thresh[:], op=Alu.is_gt)
            maskc = sp.tile([P, RB], f32)
            nc.vector.tensor_tensor(out=maskc[:], in0=m1[:], in1=m2[:], op=Alu.mult)
            # cosm = cosm + mask * (CE - cosm)
            dce = sp.tile([P, RB], f32)
            nc.vector.tensor_scalar(
                out=dce[:], in0=cosm[:], scalar1=-1.0, scalar2=CE,
                op0=Alu.mult, op1=Alu.add,
            )
            mce = sp.tile([P, RB], f32)
            nc.vector.tensor_tensor(out=mce[:], in0=maskc[:], in1=dce[:], op=Alu.mult)
            nc.vector.tensor_tensor(out=cosm[:], in0=cosm[:], in1=mce[:], op=Alu.add)

            # fixv = S*(clip(cosm, -ce, ce) - M - M*ms)
            v = sp.tile([P, RB], f32)
            nc.vector.tensor_scalar(
                out=v[:], in0=cosm[:], scalar1=-CE, scalar2=CE,
                op0=Alu.max, op1=Alu.min,
            )
            q = sp.tile([P, RB], f32)
            nc.vector.tensor_scalar(
                out=q[:], in0=v[:], scalar1=S, scalar2=-S * M,
                op0=Alu.mult, op1=Alu.add,
            )
            r_ = sp.tile([P, RB], f32)
            nc.vector.tensor_scalar(
                out=r_[:], in0=ms[:], scalar1=S * M, scalar2=None, op0=Alu.mult
            )
            fixv = sp.tile([P, RB], bf16)
            nc.vector.tensor_tensor(out=fixv[:], in0=q[:], in1=r_[:], op=Alu.subtract)

            # ---- streaming bulk pass: out = min(x, ce) * S  (bf16 in/out;
            # the DVE computes in fp32 internally with f32 immediates).
            # Half-width tiles keep the load/compute/store pipeline smooth.
            HT = CS // 2
            scattered = [False] * k_cols
            for rb in range(RB):
                rows = slice(rb * P, (rb + 1) * P)
                for h in range(2):
                    cols = slice(h * HT, (h + 1) * HT)
                    t = stp.tile([P, HT], bf16)
                    nc.sync.dma_start(out=t[:], in_=cos_bf.ap()[rows, cols])
                    nc.vector.tensor_scalar(
                        out=t[:], in0=t[:], scalar1=CE, scalar2=S,
                        op0=Alu.min, op1=Alu.mult,
                    )
                    nc.scalar.dma_start(out=out_t.ap()[rows, cols], in_=t[:])

                # Scatter column j as soon as every row-block it touches has
                # been stored.  Its out AP is a row-prefix of the output
                # (offset 0, as the indirect DMA requires) that covers
                # exactly blocks [0, blocks[j]) — Tile's dependency tracking
                # then releases it here instead of after the last store.
                for j in range(k_cols):
                    if not scattered[j] and blocks[j] == rb + 1:
                        scattered[j] = True
                        nc.gpsimd.indirect_dma_start(
                            out=out_t.ap()[0 : blocks[j] * P, :],
                            out_offset=bass.IndirectOffsetOnAxis(
                                ap=off[:, j : j + 1], axis=1
                            ),
                            in_=fixv[:, j : j + 1],
                            in_offset=None,
                            bounds_check=blocks[j] * P * CS - 1,
                            oob_is_err=False,
                        )
            assert all(scattered)

    nc.compile()
    return nc


def _get_compiled(k_cols, blocks):
    key = (k_cols, tuple(blocks))
    if key not in _COMPILED:
        _COMPILED[key] = _build(k_cols, tuple(blocks))
    return _COMPILED[key]


def _make_in_maps(cosine, norms, label):
    """Shard cosine over C; build per-core [128, 16] tables of norms and
    flat gather/scatter offsets.  Rows are permuted into slots (p, j)
    (slot -> row mapping is free: batch stats are order-invariant) such
    that owned rows occupy the lowest slot columns; returns the number of
    columns k_cols the kernel must gather/scatter."""
    cos = np.ascontiguousarray(np.asarray(cosine, dtype=np.float32))
    nr = np.asarray(norms, dtype=np.float32).reshape(-1)
    lab = np.asarray(label).astype(np.int64).reshape(-1)
    assert cos.shape == (N, C) and nr.shape == (N,) and lab.shape == (N,)

    rows = np.arange(N, dtype=np.int64)
    owned_per_core = []
    for i in range(NCORES):
        c0 = i * CS
        owned_per_core.append(
            (lab != -1) & (lab >= c0) & (lab < c0 + CS)
        )

    # Choose k_cols row-range slices (block-aligned) such that no core has
    # more than 128 owned rows in any slice.  Scatter column j then only
    # depends on the stores of the first blocks[j] row-blocks.
    k_cols = 2
    while True:
        bounds = [-(-RB * (j + 1) // k_cols) * P for j in range(k_cols)]  # row hi per slice
        lo = 0
        ok = True
        for hi in bounds:
            for owned in owned_per_core:
                if int(owned[lo:hi].sum()) > P:
                    ok = False
                    break
            if not ok:
                break
            lo = hi
        if ok or k_cols >= RB:
            break
        k_cols += 1
    blocks = tuple(b // P for b in bounds)

    in_maps = []
    for i in range(NCORES):
        c0 = i * CS
        owned = owned_per_core[i]
        # column j: owned rows in [bounds[j-1], bounds[j]) + non-owned filler
        cols = []
        fillers = list(rows[~owned][::-1])
        lo = 0
        for hi in bounds:
            got = list(rows[owned & (rows >= lo) & (rows < hi)])
            assert len(got) <= P
            while len(got) < P:
                got.append(int(fillers.pop()))
            cols.append(got)
            lo = hi
        used = set()
        for cgot in cols:
            used.update(cgot)
        rest = [int(r) for r in rows if int(r) not in used]
        perm = np.array([r for cgot in cols for r in cgot] + rest, dtype=np.int64)
        assert len(perm) == N

        offv = np.where(
            owned[perm], perm * CS + (lab[perm] - c0), np.int64(SENTINEL)
        ).astype(np.int32)
        # slot (p, j) = permuted position j*128 + p  ->  table[p, j]
        off_tab = np.ascontiguousarray(offv.reshape(RB, P).T)
        norms_tab = np.ascontiguousarray(nr[perm].reshape(RB, P).T)
        cos_slice = np.ascontiguousarray(cos[:, c0 : c0 + CS])
        in_maps.append(
            {
                "cosine": cos_slice,
                "cosine_bf": cos_slice.astype(ml_dtypes.bfloat16),
                "norms_t": norms_tab,
                "off": off_tab,
            }
        )
    return in_maps, k_cols, blocks


def _run(in_maps, k_cols, blocks, trace=False, **kwargs):
    import sys

    if "/opt/trn_rl_repo" not in sys.path:
        sys.path.insert(0, "/opt/trn_rl_repo")
    from concourse.bass_utils import run_bass_kernel_spmd

    nc = _get_compiled(k_cols, blocks)
    return run_bass_kernel_spmd(
        nc, in_maps, core_ids=list(range(NCORES)), trace=trace, **kwargs
    )


def kernel(cosine, norms, label):
    in_maps, k_cols, blocks = _make_in_maps(cosine, norms, label)
    res = _run(in_maps, k_cols, blocks)
    outs = [np.asarray(res.results[i]["out"]) for i in range(NCORES)]
    return np.concatenate(outs, axis=1).astype(np.float32)
